# revision 1
# baseline (speedup 1.0000x reference)
"""Trainium2 Bass kernel for nn_Densenet_with_skip (gauss blur -> sobel ->
angle-binned 8-direction NMS -> gate).

Reformulation (validated vs reference at 1.7e-7 in fp32 numpy):
  b  = gauss5x5(x)                      (zero pad; separable, rank-1)
  gx = (Bv@Sv)^T x (Bh@Dh)              (composed 7-tap bands, replicate pad on b)
  gy = (Bv@Dv)^T x (Bh@Sh)
  bin: m0 = (t1*|gx| >= |gy|)  -> horizontal pair
       m2 = (t1*|gy| >  |gx|)  -> vertical pair
       else diag: gx*gy<0 -> anti-diag pair, else main-diag pair
  out = x * relu(cen*b + v*max(pair))   (pair neighbors, 0 outside image)

All convolutions are done on the TensorEngine as banded matmuls in fp16
(single sweep, ~3.1e-3 rel l2 vs reference). Vertical-pair access is done by
producing row-shifted copies of b (bup/bdn) straight out of the second
(horizontal) matmul pass by shifting the stationary operand's row slice, so
the elementwise stage only ever needs free-dim shifts.
"""

import sys

import numpy as np

sys.path.insert(0, "/opt/trn_rl_repo")

import concourse.bacc as bacc
import concourse.mybir as mybir
from concourse import tile
from concourse.bass_utils import run_bass_kernel_spmd

N = 512
B_TOTAL = 32
N_CORES = 8
B_CORE = B_TOTAL // N_CORES  # 4 images per core
NCHUNK = N // 128  # 4

F16 = mybir.dt.float16
U8 = mybir.dt.uint8
F32 = mybir.dt.float32

T1 = float(np.tan(np.pi / 8))  # tan(22.5 deg)


def _band_ranges(halo):
    """Output-col range [lo, hi) per 128-chunk for a (2*halo+1)-tap band."""
    out = []
    for r in range(NCHUNK):
        lo = max(0, 128 * r - halo)
        hi = min(N, 128 * r + 128 + halo)
        out.append((lo, hi))
    return out


def _banded_mm(nc, psum_ap, lhsT_sl, w_chunks, ranges, out_poff=0):
    """Accumulate sum_r lhsT_r.T @ W_r[:, band] into psum with correct
    PSUM zero-region start/stop handling (each matmul touches either
    all-fresh or all-covered columns)."""
    covered = 0
    n = len(ranges)
    for r in range(n):
        lo, hi = ranges[r]
        first = r == 0
        last = r == n - 1
        if not first and lo < covered:
            nc.tensor.matmul(
                psum_ap[:, lo:covered],
                lhsT_sl[r],
                w_chunks[r][:, lo:covered],
                start=False,
                stop=False,
            )
            lo = covered
        nc.tensor.matmul(
            psum_ap[:, lo:hi],
            lhsT_sl[r],
            w_chunks[r][:, lo:hi],
            start=first,
            stop=last,
        )
        covered = hi


def _banded_mm2(nc, psum_ap, lhsT_a, wa, lhsT_b, wb, ranges):
    covered = 0
    n = len(ranges)
    for r in range(n):
        lo, hi = ranges[r]
        first = r == 0
        if not first and lo < covered:
            nc.tensor.matmul(psum_ap[:, lo:covered], lhsT_a[r],
                             wa[r][:, lo:covered], start=False, stop=False)
            lo = covered
        nc.tensor.matmul(psum_ap[:, lo:hi], lhsT_a[r], wa[r][:, lo:hi],
                         start=first, stop=False)
        covered = hi
    for r in range(n):
        lo, hi = ranges[r]
        last = r == n - 1
        nc.tensor.matmul(psum_ap[:, lo:hi], lhsT_b[r], wb[r][:, lo:hi],
                         start=False, stop=last)


def build_nc(cen, v):
    """Build + compile the per-core program. cen/v: nms kernel center and tap."""
    s = -v  # S = s*b ; z = (cen/s)*S - max(pair of S-neighbors)
    zs = cen / s

    nc = bacc.Bacc("TRN2", target_bir_lowering=False, debug=False)

    x_d = nc.dram_tensor("x", [B_CORE * N, N], F32, kind="ExternalInput").ap()
    w_names = ["bv", "bh", "mvx", "mhx", "mvy", "mhy"]
    w_d = {
        k: nc.dram_tensor("w_" + k, [N, N], F16, kind="ExternalInput").ap()
        for k in w_names
    }
    out_d = nc.dram_tensor("out", [B_CORE * N, N], F32, kind="ExternalOutput").ap()

    r5 = _band_ranges(2)  # 5-tap gauss band
    r7 = _band_ranges(3)  # 7-tap composed sobel band

    with tile.TileContext(nc) as tc:
        with (
            tc.tile_pool(name="wpool", bufs=1) as wpool,
            tc.tile_pool(name="xpool", bufs=1) as xpool,
            tc.tile_pool(name="tT", bufs=2) as tTpool,
            tc.tile_pool(name="post", bufs=2) as post,
            tc.tile_pool(name="outp", bufs=3) as outp,
            tc.tile_pool(name="psumv", bufs=2, space="PSUM") as psumv,
            tc.tile_pool(name="psumh", bufs=2, space="PSUM") as psumh,
        ):
            zrow = wpool.tile([1, N + 2], F16, tag="zrow")
            nc.vector.memset(zrow[:], 0.0)

            # --- load weights (band matrices), 4 row-chunks each ---
            w_sb = {}
            for k in w_names:
                w_sb[k] = []
                for r in range(NCHUNK):
                    t = wpool.tile([128, N], F16, tag=f"w_{k}_{r}")
                    nc.sync.dma_start(out=t[:], in_=w_d[k][128 * r : 128 * (r + 1), :])
                    w_sb[k].append(t)

            # --- load x as fp16 (DMA cast); quad layout [128, 4*512] ---
            xh = []
            for i in range(B_CORE):
                t = xpool.tile([128, NCHUNK * N], F16, tag=f"xh_{i}")
                for r in range(NCHUNK):
                    nc.gpsimd.dma_start(
                        out=t[:, N * r : N * (r + 1)],
                        in_=x_d[i * N + 128 * r : i * N + 128 * (r + 1), :],
                    )
                xh.append(t)

            for i in range(B_CORE):
                # ---------- vertical passes: tT[k][c] = (W_v^T x)^T ----------
                tT = {}
                for k, wname, rr in (
                    ("b", "bv", r5),
                    ("x", "mvx", r7),
                    ("y", "mvy", r7),
                ):
                    tT[k] = []
                    for c in range(NCHUNK):
                        pv = psumv.tile([128, N], F32, tag="pv")
                        lhsT = [xh[i][:, N * r + 128 * c : N * r + 128 * (c + 1)] for r in range(NCHUNK)]
                        _banded_mm(nc, pv, lhsT, w_sb[wname], rr)
                        st = tTpool.tile([128, N], F16, tag=f"tT_{k}_{c}")
                        nc.scalar.activation(
                            st[:], pv[:], mybir.ActivationFunctionType.Copy
                        )
                        tT[k].append(st)

                # ---------- horizontal passes (per row-tile) + quad elementwise ----------
                Sq = post.tile([128, NCHUNK * (N + 2)], F16, tag="S")
                Upq = post.tile([128, NCHUNK * (N + 2)], F16, tag="Up")
                Dnq = post.tile([128, NCHUNK * (N + 2)], F16, tag="Dn")
                t3 = Sq[:].rearrange("p (q w) -> p q w", w=N + 2)
                nc.vector.memset(t3[:, :, 0:1], 0.0)
                nc.vector.memset(t3[:, :, N + 1 : N + 2], 0.0)
                axq = post.tile([128, NCHUNK * N], F16, tag="ax")
                ayq = post.tile([128, NCHUNK * N], F16, tag="ay")
                pabsq = post.tile([128, NCHUNK * N], F16, tag="pabs")

                for rt in range(NCHUNK):
                    row0 = 128 * rt

                    def hpass(key, wname, rr, shift, tag):
                        p = psumh.tile([128, N], F32, tag=tag)
                        lo = row0 + shift
                        lhsT = [tT[key][c][:, lo : lo + 128] for c in range(NCHUNK)]
                        _banded_mm(nc, p, lhsT, w_sb[wname], rr)
                        return p

                    pb = hpass("b", "bh", r5, 0, "pb")
                    pgx = hpass("x", "mhx", r7, 0, "pgx")
                    # pgy holds gy; its accumulation group stays open. After
                    # the |gy| extract, one extra sweep adds gx into the same
                    # psum (sign test: sign(gx*gy)<0 iff |gx+gy| < sqrt2*|gx|).
                    pgy = psumh.tile([128, N], F32, tag="pgy")
                    lhsT_y = [tT["y"][c][:, row0 : row0 + 128] for c in range(NCHUNK)]
                    covered = 0
                    for r in range(NCHUNK):
                        lo2, hi2 = r7[r]
                        first = r == 0
                        if not first and lo2 < covered:
                            nc.tensor.matmul(pgy[:, lo2:covered], lhsT_y[r],
                                             w_sb["mhy"][r][:, lo2:covered],
                                             start=False, stop=False)
                            lo2 = covered
                        nc.tensor.matmul(pgy[:, lo2:hi2], lhsT_y[r],
                                         w_sb["mhy"][r][:, lo2:hi2],
                                         start=first, stop=False)
                        covered = hi2

                    q0 = rt * (N + 2)
                    f0 = rt * N
                    nc.scalar.activation(
                        Sq[:, q0 + 1 : q0 + N + 1], pb[:],
                        mybir.ActivationFunctionType.Copy, scale=s,
                    )
                    nc.scalar.activation(
                        axq[:, f0 : f0 + N], pgx[:],
                        mybir.ActivationFunctionType.Abs,
                    )
                    nc.scalar.activation(
                        ayq[:, f0 : f0 + N], pgy[:],
                        mybir.ActivationFunctionType.Abs,
                    )
                    lhsT_x = [tT["x"][c][:, row0 : row0 + 128] for c in range(NCHUNK)]
                    for r in range(NCHUNK):
                        lo2, hi2 = r7[r]
                        nc.tensor.matmul(pgy[:, lo2:hi2], lhsT_x[r],
                                         w_sb["mhx"][r][:, lo2:hi2],
                                         start=False, stop=(r == NCHUNK - 1))
                    nc.scalar.activation(
                        pabsq[:, f0 : f0 + N], pgy[:],
                        mybir.ActivationFunctionType.Abs,
                    )

                # Up/Dn = partition-shifted copies of S via DMA (incl pads)
                W2 = N + 2
                S3v = Sq[:].rearrange("p (q w) -> p q w", w=W2)
                U3v = Upq[:].rearrange("p (q w) -> p q w", w=W2)
                D3v = Dnq[:].rearrange("p (q w) -> p q w", w=W2)
                # Up[p, q] = S[p+1, q]; Up[127, q] = S[0, q+1]; Up[127, 3] = 0
                nc.sync.dma_start(out=U3v[0:127, :, :], in_=S3v[1:128, :, :])
                nc.sync.dma_start(out=U3v[127:128, 0:3, :], in_=S3v[0:1, 1:4, :])
                nc.sync.dma_start(
                    out=U3v[127:128, 3:4, :],
                    in_=zrow[:].rearrange("p (q w) -> p q w", w=W2),
                )
                # Dn[p, q] = S[p-1, q]; Dn[0, q] = S[127, q-1]; Dn[0, 0] = 0
                nc.sync.dma_start(out=D3v[1:128, :, :], in_=S3v[0:127, :, :])
                nc.sync.dma_start(out=D3v[0:1, 1:4, :], in_=S3v[127:128, 0:3, :])
                nc.sync.dma_start(
                    out=D3v[0:1, 0:1, :],
                    in_=zrow[:].rearrange("p (q w) -> p q w", w=W2),
                )
                S3 = Sq[:].rearrange("p (q w) -> p q w", w=W2)
                U3 = Upq[:].rearrange("p (q w) -> p q w", w=W2)
                D3 = Dnq[:].rearrange("p (q w) -> p q w", w=W2)
                ax3 = axq[:].rearrange("p (q w) -> p q w", w=N)
                ay3 = ayq[:].rearrange("p (q w) -> p q w", w=N)
                pa3 = pabsq[:].rearrange("p (q w) -> p q w", w=N)

                def qt(tag, dt=F16):
                    t = post.tile([128, NCHUNK * N], dt, tag=tag)
                    return t, t[:].rearrange("p (q w) -> p q w", w=N)

                selq, sel3 = qt("sel")
                pm3q, pm33 = qt("pm3")
                pm2q, pm23 = qt("pm2")
                pm0q, pm03 = qt("pm0")
                mnegq, mneg3 = qt("mneg", U8)
                m0q, m03 = qt("m0", U8)
                m2q, m23 = qt("m2", U8)
                zq, z3 = qt("z")

                TT = nc.vector.tensor_tensor
                TT(out=sel3, in0=D3[:, :, 0:N], in1=U3[:, :, 2 : N + 2],
                   op=mybir.AluOpType.max)
                TT(out=pm33, in0=D3[:, :, 2 : N + 2], in1=U3[:, :, 0:N],
                   op=mybir.AluOpType.max)
                TT(out=pm23, in0=D3[:, :, 1 : N + 1], in1=U3[:, :, 1 : N + 1],
                   op=mybir.AluOpType.max)
                TT(out=pm03, in0=S3[:, :, 0:N], in1=S3[:, :, 2 : N + 2],
                   op=mybir.AluOpType.max)
                nc.vector.scalar_tensor_tensor(
                    out=mneg3, in0=ax3, scalar=float(np.sqrt(2.0)), in1=pa3,
                    op0=mybir.AluOpType.mult, op1=mybir.AluOpType.is_gt,
                )
                nc.vector.scalar_tensor_tensor(
                    out=m03, in0=ax3, scalar=T1, in1=ay3,
                    op0=mybir.AluOpType.mult, op1=mybir.AluOpType.is_ge,
                )
                nc.vector.scalar_tensor_tensor(
                    out=m23, in0=ay3, scalar=T1, in1=ax3,
                    op0=mybir.AluOpType.mult, op1=mybir.AluOpType.is_gt,
                )
                nc.vector.copy_predicated(sel3, mneg3, pm33)
                nc.vector.copy_predicated(sel3, m23, pm23)
                nc.vector.copy_predicated(sel3, m03, pm03)
                oq = outp.tile([128, NCHUNK * N], F16, tag="o")
                o3 = oq[:].rearrange("p (q w) -> p q w", w=N)
                x3 = xh[i][:].rearrange("p (q w) -> p q w", w=N)
                for h0 in (0, 2):
                    hs = slice(h0, h0 + 2)
                    nc.vector.scalar_tensor_tensor(
                        out=z3[:, hs], in0=S3[:, hs, 1 : N + 1], scalar=zs,
                        in1=sel3[:, hs],
                        op0=mybir.AluOpType.mult, op1=mybir.AluOpType.subtract,
                    )
                    nc.vector.scalar_tensor_tensor(
                        out=o3[:, hs], in0=z3[:, hs], scalar=0.0, in1=x3[:, hs],
                        op0=mybir.AluOpType.max, op1=mybir.AluOpType.mult,
                    )
                    for rt in range(h0, h0 + 2):
                        nc.gpsimd.dma_start(
                            out=out_d[i * N + 128 * rt : i * N + 128 * (rt + 1), :],
                            in_=oq[:, N * rt : N * (rt + 1)],
                        )

    nc.compile()
    return nc


# ---------------------------------------------------------------------------
# host side
# ---------------------------------------------------------------------------

def _make_band(weights, offsets, pad):
    M = np.zeros((N, N), dtype=np.float64)
    for w, o in zip(weights, offsets):
        idx = np.arange(N)
        src = idx + o
        if pad == "replicate":
            np.add.at(M, (np.clip(src, 0, N - 1), idx), w)
        else:
            ok = (src >= 0) & (src < N)
            np.add.at(M, (src[ok], idx[ok]), w)
    return M


def _host_weights(gauss_kernel):
    gk = np.asarray(gauss_kernel, dtype=np.float64)[0, 0]
    U, sv, Vt = np.linalg.svd(gk)
    assert sv[1] < 1e-5 * sv[0], "gauss kernel not rank-1 separable"
    wv = U[:, 0] * np.sqrt(sv[0])
    wh = Vt[0] * np.sqrt(sv[0])
    if wv.sum() < 0:
        wv, wh = -wv, -wh
    o5 = [-2, -1, 0, 1, 2]
    o3 = [-1, 0, 1]
    Bv = _make_band(wv, o5, "zero")
    Bh = _make_band(wh, o5, "zero")
    Sv = _make_band([1, 2, 1], o3, "replicate")
    Dv = _make_band([-1, 0, 1], o3, "replicate")
    Sh = _make_band([1, 2, 1], o3, "replicate")
    Dh = _make_band([-1, 0, 1], o3, "replicate")
    f16 = lambda a: np.ascontiguousarray(a, dtype=np.float16)
    return {
        "w_bv": f16(Bv),
        "w_bh": f16(Bh),
        "w_mvx": f16(Bv @ Sv),
        "w_mhx": f16(Bh @ Dh),
        "w_mvy": f16(Bv @ Dv),
        "w_mhy": f16(Bh @ Sh),
    }


_NC_CACHE = {}
LAST_RESULT = None


def kernel(reconst, gauss_kernel, nms_kernel):
    nk = np.asarray(nms_kernel, dtype=np.float64)
    cen = float(nk[0, 0, 1, 1])
    v = float(nk[0, 0, 1, 2])
    # verify nms kernel structure: center + single tap v per direction
    pos = [(1, 2), (2, 2), (2, 1), (2, 0), (1, 0), (0, 0), (0, 1), (0, 2)]
    for d, (r, c) in enumerate(pos):
        k = nk[d, 0].copy()
        assert abs(k[1, 1] - cen) < 1e-6 and abs(k[r, c] - v) < 1e-6
        k[1, 1] = 0.0
        k[r, c] = 0.0
        assert np.abs(k).max() < 1e-7
    assert v < 0

    key = (round(cen, 9), round(v, 9))
    if key not in _NC_CACHE:
        _NC_CACHE[key] = build_nc(cen, v)
    nc = _NC_CACHE[key]

    w = _host_weights(gauss_kernel)
    x = np.asarray(reconst, dtype=np.float32).reshape(B_TOTAL, N, N)
    in_maps = []
    for core in range(N_CORES):
        m = {"x": np.ascontiguousarray(
            x[core * B_CORE : (core + 1) * B_CORE].reshape(B_CORE * N, N)
        )}
        m.update(w)
        in_maps.append(m)

    res = run_bass_kernel_spmd(nc, in_maps, core_ids=list(range(N_CORES)))
    global LAST_RESULT
    LAST_RESULT = res
    out = np.concatenate(
        [r["out"].reshape(B_CORE, 1, N, N) for r in res.results], axis=0
    )
    return out.astype(np.float32)



# revision 8
# speedup vs baseline: 1.1805x; 1.1805x over previous
"""Trainium2 Bass kernel for nn_Densenet_with_skip (gauss blur -> sobel ->
angle-binned 8-direction NMS -> gate).

Reformulation (validated vs reference at ~2.7e-3 rel l2 in f16 numpy):
  b  = gauss5x5(x)                      (zero pad; separable, rank-1)
  gx = (Bv@Sv)^T x (Bh@Dh)              (composed 7-tap bands, replicate pad)
  gy = (Bv@Dv)^T x (Bh@Sh)
  masks via squared comparisons (all legal TRN2 ops):
    m0   = T1^2 gx^2 >= gy^2            -> horizontal pair
    m2   = T1^4 gy^2 >  T1^2 gx^2      -> vertical pair
    mneg = 2 gx^2    >  (gx+gy)^2      -> anti-diagonal pair (else main diag)
  out = relu(x*(S - sel/zs))*zs,  S = s*b, sel = max of selected pair of S
  (x >= 0 so x*relu(.) == relu(x*.))

Engine placement: Act does all PSUM extracts (tT copies, S, and the three
Square extracts feeding the masks); DVE does mask compares (TT 2x), pair
maxes (TT 2x), the copy_predicated select chain, and z/gate; Pool does
the tensor_scalar rescales (syT, sxT2, hsel) plus DMA descriptor gen.
The (gx+gy)^2 input reuses the gy PSUM accumulation group: after gy^2 is
extracted, one extra banded sweep adds gx into the same PSUM bank.
"""

import sys

import numpy as np

sys.path.insert(0, "/opt/trn_rl_repo")

import concourse.bacc as bacc
import concourse.mybir as mybir
from concourse import tile
from concourse.bass_utils import run_bass_kernel_spmd

N = 512
B_TOTAL = 32
N_CORES = 8
B_CORE = B_TOTAL // N_CORES  # 4 images per core
NCHUNK = N // 128  # 4
WBAND = 136  # padded band width per weight chunk

F16 = mybir.dt.float16
U16 = mybir.dt.uint16
F32 = mybir.dt.float32

T1 = float(np.tan(np.pi / 8))  # tan(22.5 deg)
W_NAMES = ["bv", "bh", "mvx", "mhx", "mvy", "mhy"]


def _band_ranges(halo):
    """Output-col range [lo, hi) per 128-chunk for a (2*halo+1)-tap band."""
    out = []
    for r in range(NCHUNK):
        lo = max(0, 128 * r - halo)
        hi = min(N, 128 * r + 128 + halo)
        out.append((lo, hi))
    return out


def _banded_mm(nc, psum_ap, lhsT_sl, w_slices, ranges, stop=True):
    """Accumulate sum_r lhsT_r.T @ W_r[:, band] into psum. w_slices[r] is
    (ap, lo0): the band-trimmed stationary chunk and its global col offset."""
    covered = 0
    n = len(ranges)
    for r in range(n):
        lo, hi = ranges[r]
        wap, lo0 = w_slices[r]
        first = r == 0
        last = stop and r == n - 1
        if not first and lo < covered:
            nc.tensor.matmul(
                psum_ap[:, lo:covered],
                lhsT_sl[r],
                wap[:, lo - lo0 : covered - lo0],
                start=False,
                stop=False,
            )
            lo = covered
        nc.tensor.matmul(
            psum_ap[:, lo:hi],
            lhsT_sl[r],
            wap[:, lo - lo0 : hi - lo0],
            start=first,
            stop=last,
        )
        covered = hi


def build_nc(cen, v):
    """Build + compile the per-core program. cen/v: nms kernel center/tap."""
    s = -v  # S = s*b
    zs = cen / s  # z = zs*S - sel ; out = x * relu(z)

    nc = bacc.Bacc("TRN2", target_bir_lowering=False, debug=False)

    x_d = nc.dram_tensor("x", [B_CORE * N, N], F32, kind="ExternalInput").ap()
    w_d = nc.dram_tensor(
        "wq", [128, len(W_NAMES) * NCHUNK * WBAND], F16, kind="ExternalInput"
    ).ap()
    out_d = nc.dram_tensor("out", [B_CORE * N, N], F32, kind="ExternalOutput").ap()

    r5 = _band_ranges(2)
    r7 = _band_ranges(3)
    RR = {"bv": r5, "bh": r5, "mvx": r7, "mhx": r7, "mvy": r7, "mhy": r7}

    SQ = mybir.ActivationFunctionType.Square
    CPY = mybir.ActivationFunctionType.Copy

    with tile.TileContext(nc) as tc:
        with (
            tc.tile_pool(name="wpool", bufs=1) as wpool,
            tc.tile_pool(name="xpool", bufs=3) as xpool,
            tc.tile_pool(name="tT", bufs=2) as tTpool,
            tc.tile_pool(name="post", bufs=2) as post,
            tc.tile_pool(name="outp", bufs=2) as outp,
            tc.tile_pool(name="psumv", bufs=1, space="PSUM") as psumv,
            tc.tile_pool(name="psb", bufs=2, space="PSUM") as psb,
            tc.tile_pool(name="psx", bufs=1, space="PSUM") as psx,
            tc.tile_pool(name="psy", bufs=1, space="PSUM") as psy,
        ):
            zrow = wpool.tile([1, N + 2], F16, tag="zrow")
            nc.vector.memset(zrow[:], 0.0)

            # --- all weights in one DMA; slice band chunks per (k, r) ---
            wq = wpool.tile([128, len(W_NAMES) * NCHUNK * WBAND], F16, tag="wq")
            nc.sync.dma_start(out=wq[:], in_=w_d[:, :])
            w_sl = {}
            for ki, k in enumerate(W_NAMES):
                w_sl[k] = []
                for r in range(NCHUNK):
                    c0 = (ki * NCHUNK + r) * WBAND
                    lo0 = RR[k][r][0]
                    w_sl[k].append((wq[:, c0 : c0 + WBAND], lo0))

            W2 = N + 2
            for i in range(B_CORE):
                # ---- load image i as f16, quad layout [128, 4*512], 1 desc ----
                xq = xpool.tile([128, NCHUNK * N], F16, tag="xq")
                nc.gpsimd.dma_start(
                    out=xq[:].rearrange("p (q w) -> p q w", w=N),
                    in_=x_d[i * N : (i + 1) * N, :].rearrange(
                        "(q p) w -> p q w", p=128
                    ),
                )

                # ---- vertical passes: tT[k] quad = (W_v^T x)^T per col-chunk ----
                tT = {}
                for k, wname in (("b", "bv"), ("x", "mvx"), ("y", "mvy")):
                    tq = tTpool.tile([128, NCHUNK * N], F16, tag=f"tT_{k}")
                    pv = psumv.tile([128, NCHUNK * N], F32, tag="pv")
                    for c in range(NCHUNK):
                        lhsT = [
                            xq[:, N * r + 128 * c : N * r + 128 * (c + 1)]
                            for r in range(NCHUNK)
                        ]
                        _banded_mm(
                            nc, pv[:, N * c : N * (c + 1)], lhsT,
                            w_sl[wname], RR[wname],
                        )
                    nc.scalar.activation(tq[:], pv[:], CPY)
                    tT[k] = tq

                # ---- horizontal passes + extracts ----
                Sq = post.tile([128, NCHUNK * W2], F16, tag="S")
                S3 = Sq[:].rearrange("p (q w) -> p q w", w=W2)
                nc.gpsimd.memset(S3[:, :, 0:1], 0.0)
                nc.gpsimd.memset(S3[:, :, N + 1 : N + 2], 0.0)
                sxT = post.tile([128, NCHUNK * N], F16, tag="sxT")
                syq = post.tile([128, NCHUNK * N], F16, tag="sy")
                ssq = post.tile([128, NCHUNK * N], F16, tag="ss")

                Upq = post.tile([128, NCHUNK * W2], F16, tag="Up")
                Dnq = post.tile([128, NCHUNK * W2], F16, tag="Dn")
                U3 = Upq[:].rearrange("p (q w) -> p q w", w=W2)
                D3 = Dnq[:].rearrange("p (q w) -> p q w", w=W2)
                zr3 = zrow[:].rearrange("p (q w) -> p q w", w=W2)
                TT = nc.vector.tensor_tensor

                # process in image halves (2 row-chunks each) so the DVE
                # elementwise tail of the last image is half as long and the
                # first image's elementwise work starts earlier.
                for h in range(2):
                    for rt in (2 * h, 2 * h + 1):
                        row0 = 128 * rt

                        def lhs(key):
                            return [
                                tT[key][:, N * c + row0 : N * c + row0 + 128]
                                for c in range(NCHUNK)
                            ]

                        pb = psb.tile([128, N], F32, tag="pb")
                        _banded_mm(nc, pb, lhs("b"), w_sl["bh"], r5)
                        pgx = psx.tile([128, N], F32, tag="pgx")
                        _banded_mm(nc, pgx, lhs("x"), w_sl["mhx"], r7)
                        # pgy accumulation stays open: after gy^2 extract,
                        # one more sweep adds gx in-bank for (gx+gy)^2.
                        pgy = psy.tile([128, N], F32, tag="pgy")
                        _banded_mm(nc, pgy, lhs("y"), w_sl["mhy"], r7, stop=False)

                        q0 = rt * W2
                        f0 = rt * N
                        nc.scalar.activation(
                            Sq[:, q0 + 1 : q0 + N + 1], pb[:], CPY, scale=s
                        )
                        nc.scalar.activation(
                            sxT[:, f0 : f0 + N], pgx[:], SQ, scale=T1
                        )
                        nc.scalar.activation(syq[:, f0 : f0 + N], pgy[:], SQ)
                        for r in range(NCHUNK):
                            lo2, hi2 = r7[r]
                            wap, lo0 = w_sl["mhx"][r]
                            nc.tensor.matmul(
                                pgy[:, lo2:hi2],
                                lhs("x")[r],
                                wap[:, lo2 - lo0 : hi2 - lo0],
                                start=False,
                                stop=(r == NCHUNK - 1),
                            )
                        nc.scalar.activation(ssq[:, f0 : f0 + N], pgy[:], SQ)

                    # ---- per-half elementwise ----
                    fh = slice(2 * h * N, (2 * h + 2) * N)
                    qs = slice(2 * h, 2 * h + 2)
                    S3h = S3[:, qs, :]
                    U3h = U3[:, qs, :]
                    D3h = D3[:, qs, :]

                    # masks (Pool rescales off critical path + DVE TT 2x)
                    syT = post.tile([128, 2 * N], F16, tag=f"syT{h}")
                    sxT2 = post.tile([128, 2 * N], F16, tag=f"sxT2{h}")
                    nc.gpsimd.tensor_scalar(
                        out=syT[:], in0=syq[:, fh], scalar1=T1 ** 4,
                        scalar2=None, op0=mybir.AluOpType.mult,
                    )
                    nc.gpsimd.tensor_scalar(
                        out=sxT2[:], in0=sxT[:, fh], scalar1=2.0 / T1 ** 2,
                        scalar2=None, op0=mybir.AluOpType.mult,
                    )
                    m0q = post.tile([128, 2 * N], F16, tag=f"m0{h}")
                    m2q = post.tile([128, 2 * N], F16, tag=f"m2{h}")
                    mnq = post.tile([128, 2 * N], F16, tag=f"mneg{h}")
                    TT(out=m0q[:], in0=sxT[:, fh], in1=syq[:, fh],
                       op=mybir.AluOpType.is_ge)
                    TT(out=m2q[:], in0=syT[:], in1=sxT[:, fh],
                       op=mybir.AluOpType.is_gt)
                    TT(out=mnq[:], in0=sxT2[:], in1=ssq[:, fh],
                       op=mybir.AluOpType.is_gt)

                    # Up/Dn partition shifts for this half's chunks.
                    # Up[p, q] = S[p+1, q]; Up[127, q] = S[0, q+1] (0 at q=3)
                    nc.sync.dma_start(out=U3[0:127, qs, :], in_=S3[1:128, qs, :])
                    if h == 0:
                        nc.sync.dma_start(
                            out=U3[127:128, 0:2, :], in_=S3[0:1, 1:3, :]
                        )
                    else:
                        nc.sync.dma_start(
                            out=U3[127:128, 2:3, :], in_=S3[0:1, 3:4, :]
                        )
                        nc.sync.dma_start(out=U3[127:128, 3:4, :], in_=zr3)
                    # Dn[p, q] = S[p-1, q]; Dn[0, q] = S[127, q-1] (0 at q=0)
                    nc.sync.dma_start(out=D3[1:128, qs, :], in_=S3[0:127, qs, :])
                    if h == 0:
                        nc.sync.dma_start(out=D3[0:1, 0:1, :], in_=zr3)
                        nc.sync.dma_start(
                            out=D3[0:1, 1:2, :], in_=S3[127:128, 0:1, :]
                        )
                    else:
                        nc.sync.dma_start(
                            out=D3[0:1, 2:4, :], in_=S3[127:128, 1:3, :]
                        )

                    # pair maxes; sel = main-diag default
                    selq = post.tile([128, 2 * N], F16, tag=f"sel{h}")
                    sel3 = selq[:].rearrange("p (q w) -> p q w", w=N)
                    paq = post.tile([128, 2 * N], F16, tag=f"panti{h}")
                    pa3 = paq[:].rearrange("p (q w) -> p q w", w=N)
                    p2q = post.tile([128, 2 * N], F16, tag=f"pm2{h}")
                    p23 = p2q[:].rearrange("p (q w) -> p q w", w=N)
                    p0q = post.tile([128, 2 * N], F16, tag=f"pm0{h}")
                    p03 = p0q[:].rearrange("p (q w) -> p q w", w=N)

                    TT(out=sel3, in0=D3h[:, :, 0:N], in1=U3h[:, :, 2 : N + 2],
                       op=mybir.AluOpType.max)       # main diag {UL, DR}
                    TT(out=pa3, in0=D3h[:, :, 2 : N + 2], in1=U3h[:, :, 0:N],
                       op=mybir.AluOpType.max)       # anti diag {UR, DL}
                    TT(out=p23, in0=D3h[:, :, 1 : N + 1], in1=U3h[:, :, 1 : N + 1],
                       op=mybir.AluOpType.max)       # vertical {Uc, Dc}
                    TT(out=p03, in0=S3h[:, :, 0:N], in1=S3h[:, :, 2 : N + 2],
                       op=mybir.AluOpType.max)       # horizontal {Sl, Sr}

                    nc.vector.copy_predicated(selq[:], mnq[:].bitcast(U16), paq[:])
                    nc.vector.copy_predicated(selq[:], m2q[:].bitcast(U16), p2q[:])
                    nc.vector.copy_predicated(selq[:], m0q[:].bitcast(U16), p0q[:])

                    # out = relu(x*(S - sel/zs))*zs  (x >= 0)
                    hq = post.tile([128, 2 * N], F16, tag=f"hsel{h}")
                    zq = post.tile([128, 2 * N], F16, tag=f"z{h}")
                    wq2 = post.tile([128, 2 * N], F16, tag=f"wz{h}")
                    oq = outp.tile([128, 2 * N], F16, tag=f"o{h}")
                    nc.vector.tensor_scalar(
                        out=hq[:], in0=selq[:], scalar1=1.0 / zs, scalar2=None,
                        op0=mybir.AluOpType.mult,
                    )
                    z3 = zq[:].rearrange("p (q w) -> p q w", w=N)
                    TT(out=z3, in0=S3h[:, :, 1 : N + 1],
                       in1=hq[:].rearrange("p (q w) -> p q w", w=N),
                       op=mybir.AluOpType.subtract)
                    TT(out=wq2[:], in0=zq[:], in1=xq[:, fh],
                       op=mybir.AluOpType.mult)
                    nc.vector.tensor_scalar(
                        out=oq[:], in0=wq2[:], scalar1=0.0, scalar2=zs,
                        op0=mybir.AluOpType.max, op1=mybir.AluOpType.mult,
                    )

                    # store this half, 1 desc
                    nc.gpsimd.dma_start(
                        out=out_d[
                            i * N + 256 * h : i * N + 256 * (h + 1), :
                        ].rearrange("(q p) w -> p q w", p=128),
                        in_=oq[:].rearrange("p (q w) -> p q w", w=N),
                    )

    nc.compile()
    return nc


# ---------------------------------------------------------------------------
# host side
# ---------------------------------------------------------------------------

def _make_band(weights, offsets, pad):
    M = np.zeros((N, N), dtype=np.float64)
    for w, o in zip(weights, offsets):
        idx = np.arange(N)
        src = idx + o
        if pad == "replicate":
            np.add.at(M, (np.clip(src, 0, N - 1), idx), w)
        else:
            ok = (src >= 0) & (src < N)
            np.add.at(M, (src[ok], idx[ok]), w)
    return M


def _host_weights(gauss_kernel):
    gk = np.asarray(gauss_kernel, dtype=np.float64)[0, 0]
    U, sv, Vt = np.linalg.svd(gk)
    assert sv[1] < 1e-5 * sv[0], "gauss kernel not rank-1 separable"
    wv = U[:, 0] * np.sqrt(sv[0])
    wh = Vt[0] * np.sqrt(sv[0])
    if wv.sum() < 0:
        wv, wh = -wv, -wh
    o5 = [-2, -1, 0, 1, 2]
    o3 = [-1, 0, 1]
    Bv = _make_band(wv, o5, "zero")
    Bh = _make_band(wh, o5, "zero")
    Sv = _make_band([1, 2, 1], o3, "replicate")
    Dv = _make_band([-1, 0, 1], o3, "replicate")
    Sh = _make_band([1, 2, 1], o3, "replicate")
    Dh = _make_band([-1, 0, 1], o3, "replicate")
    mats = {
        "bv": Bv, "bh": Bh,
        "mvx": Bv @ Sv, "mhx": Bh @ Dh,
        "mvy": Bv @ Dv, "mhy": Bh @ Sh,
    }
    halo = {"bv": 2, "bh": 2, "mvx": 3, "mhx": 3, "mvy": 3, "mhy": 3}
    wq = np.zeros((128, len(W_NAMES) * NCHUNK * WBAND), dtype=np.float16)
    for ki, k in enumerate(W_NAMES):
        M = mats[k]
        h = halo[k]
        for r in range(NCHUNK):
            lo = max(0, 128 * r - h)
            hi = min(N, 128 * r + 128 + h)
            c0 = (ki * NCHUNK + r) * WBAND
            wq[:, c0 : c0 + (hi - lo)] = M[128 * r : 128 * (r + 1), lo:hi].astype(
                np.float16
            )
    return wq


_NC_CACHE = {}
LAST_RESULT = None


def kernel(reconst, gauss_kernel, nms_kernel):
    nk = np.asarray(nms_kernel, dtype=np.float64)
    cen = float(nk[0, 0, 1, 1])
    v = float(nk[0, 0, 1, 2])
    # verify nms kernel structure: center + single tap v per direction
    pos = [(1, 2), (2, 2), (2, 1), (2, 0), (1, 0), (0, 0), (0, 1), (0, 2)]
    for d, (r, c) in enumerate(pos):
        k = nk[d, 0].copy()
        assert abs(k[1, 1] - cen) < 1e-6 and abs(k[r, c] - v) < 1e-6
        k[1, 1] = 0.0
        k[r, c] = 0.0
        assert np.abs(k).max() < 1e-7
    assert v < 0

    key = (round(cen, 9), round(v, 9))
    if key not in _NC_CACHE:
        _NC_CACHE[key] = build_nc(cen, v)
    nc = _NC_CACHE[key]

    wq = _host_weights(gauss_kernel)
    x = np.asarray(reconst, dtype=np.float32).reshape(B_TOTAL, N, N)
    in_maps = []
    for core in range(N_CORES):
        m = {
            "x": np.ascontiguousarray(
                x[core * B_CORE : (core + 1) * B_CORE].reshape(B_CORE * N, N)
            ),
            "wq": wq,
        }
        in_maps.append(m)

    res = run_bass_kernel_spmd(nc, in_maps, core_ids=list(range(N_CORES)))
    global LAST_RESULT
    LAST_RESULT = res
    out = np.concatenate(
        [r["out"].reshape(B_CORE, 1, N, N) for r in res.results], axis=0
    )
    return out.astype(np.float32)


# revision 12
# speedup vs baseline: 1.1953x; 1.0125x over previous
"""Trainium2 Bass kernel for nn_Densenet_with_skip (gauss blur -> sobel ->
angle-binned 8-direction NMS -> gate).

Reformulation (validated vs reference at ~2.7e-3 rel l2 in f16 numpy):
  b  = gauss5x5(x)                      (zero pad; separable, rank-1)
  gx = (Bv@Sv)^T x (Bh@Dh)              (composed 7-tap bands, replicate pad)
  gy = (Bv@Dv)^T x (Bh@Sh)
  masks via squared comparisons (all legal TRN2 ops):
    m0   = T1^2 gx^2 >= gy^2            -> horizontal pair
    m2   = T1^4 gy^2 >  T1^2 gx^2      -> vertical pair
    mneg = 2 gx^2    >  (gx+gy)^2      -> anti-diagonal pair (else main diag)
  out = relu(x*(S - sel/zs))*zs,  S = s*b, sel = max of selected pair of S
  (x >= 0 so x*relu(.) == relu(x*.))

Engine placement: Act does all PSUM extracts (tT copies, S, and the three
Square extracts feeding the masks); DVE does mask compares (TT 2x), pair
maxes (TT 2x), the copy_predicated select chain, and z/gate; Pool does
the tensor_scalar rescales (syT, sxT2, hsel) plus DMA descriptor gen.
The (gx+gy)^2 input reuses the gy PSUM accumulation group: after gy^2 is
extracted, one extra banded sweep adds gx into the same PSUM bank.
"""

import sys

import numpy as np

sys.path.insert(0, "/opt/trn_rl_repo")

import concourse.bacc as bacc
import concourse.mybir as mybir
from concourse import tile
from concourse.bass_utils import run_bass_kernel_spmd

N = 512
B_TOTAL = 32
N_CORES = 8
B_CORE = B_TOTAL // N_CORES  # 4 images per core
NCHUNK = N // 128  # 4
WBAND = 136  # padded band width per weight chunk

F16 = mybir.dt.float16
U16 = mybir.dt.uint16
F32 = mybir.dt.float32

T1 = float(np.tan(np.pi / 8))  # tan(22.5 deg)
W_NAMES = ["bv", "bh", "mvx", "mhx", "mvy", "mhy"]


def _band_ranges(halo):
    """Output-col range [lo, hi) per 128-chunk for a (2*halo+1)-tap band."""
    out = []
    for r in range(NCHUNK):
        lo = max(0, 128 * r - halo)
        hi = min(N, 128 * r + 128 + halo)
        out.append((lo, hi))
    return out


def _banded_mm(nc, psum_ap, lhsT_sl, w_slices, ranges, stop=True):
    """Accumulate sum_r lhsT_r.T @ W_r[:, band] into psum. w_slices[r] is
    (ap, lo0): the band-trimmed stationary chunk and its global col offset."""
    covered = 0
    n = len(ranges)
    for r in range(n):
        lo, hi = ranges[r]
        wap, lo0 = w_slices[r]
        first = r == 0
        last = stop and r == n - 1
        if not first and lo < covered:
            nc.tensor.matmul(
                psum_ap[:, lo:covered],
                lhsT_sl[r],
                wap[:, lo - lo0 : covered - lo0],
                start=False,
                stop=False,
            )
            lo = covered
        nc.tensor.matmul(
            psum_ap[:, lo:hi],
            lhsT_sl[r],
            wap[:, lo - lo0 : hi - lo0],
            start=first,
            stop=last,
        )
        covered = hi


def build_nc(cen, v):
    """Build + compile the per-core program. cen/v: nms kernel center/tap."""
    s = -v  # S = s*b
    zs = cen / s  # z = zs*S - sel ; out = x * relu(z)

    nc = bacc.Bacc("TRN2", target_bir_lowering=False, debug=False)

    x_d = nc.dram_tensor("x", [B_CORE * N, N], F32, kind="ExternalInput").ap()
    w_d = nc.dram_tensor(
        "wq", [128, len(W_NAMES) * NCHUNK * WBAND], F16, kind="ExternalInput"
    ).ap()
    out_d = nc.dram_tensor("out", [B_CORE * N, N], F32, kind="ExternalOutput").ap()

    r5 = _band_ranges(2)
    r7 = _band_ranges(3)
    RR = {"bv": r5, "bh": r5, "mvx": r7, "mhx": r7, "mvy": r7, "mhy": r7}

    SQ = mybir.ActivationFunctionType.Square
    CPY = mybir.ActivationFunctionType.Copy

    with tile.TileContext(nc) as tc:
        with (
            tc.tile_pool(name="wpool", bufs=1) as wpool,
            tc.tile_pool(name="xpool", bufs=3) as xpool,
            tc.tile_pool(name="tT", bufs=2) as tTpool,
            tc.tile_pool(name="post", bufs=2) as post,
            tc.tile_pool(name="outp", bufs=2) as outp,
            tc.tile_pool(name="psumv", bufs=1, space="PSUM") as psumv,
            tc.tile_pool(name="psb", bufs=2, space="PSUM") as psb,
            tc.tile_pool(name="psx", bufs=1, space="PSUM") as psx,
            tc.tile_pool(name="psy", bufs=1, space="PSUM") as psy,
        ):
            zrow = wpool.tile([1, N + 2], F16, tag="zrow")
            nc.vector.memset(zrow[:], 0.0)

            # --- all weights in one DMA; slice band chunks per (k, r) ---
            wq = wpool.tile([128, len(W_NAMES) * NCHUNK * WBAND], F16, tag="wq")
            nc.sync.dma_start(out=wq[:], in_=w_d[:, :])
            w_sl = {}
            for ki, k in enumerate(W_NAMES):
                w_sl[k] = []
                for r in range(NCHUNK):
                    c0 = (ki * NCHUNK + r) * WBAND
                    lo0 = RR[k][r][0]
                    w_sl[k].append((wq[:, c0 : c0 + WBAND], lo0))

            W2 = N + 2
            for i in range(B_CORE):
                # ---- load image i as f16, quad layout, one desc per row
                # chunk so the first vertical matmuls start after chunk 0
                # lands instead of waiting for the whole 4MB transfer ----
                xq = xpool.tile([128, NCHUNK * N], F16, tag="xq")
                for r in range(NCHUNK):
                    nc.gpsimd.dma_start(
                        out=xq[:, N * r : N * (r + 1)],
                        in_=x_d[i * N + 128 * r : i * N + 128 * (r + 1), :],
                    )

                # Emission order feeds DVE as early as possible: the blur
                # pipeline (Vb -> Hb -> S -> shifts -> pair maxes) comes
                # first, then the sobel pipelines (Vx/Hx, Vy/Hy) that feed
                # the masks, then the per-half select/gate tail.
                tT = {}

                def vpass(k, wname):
                    tq = tTpool.tile([128, NCHUNK * N], F16, tag=f"tT_{k}")
                    pv = psumv.tile([128, NCHUNK * N], F32, tag="pv")
                    for c in range(NCHUNK):
                        lhsT = [
                            xq[:, N * r + 128 * c : N * r + 128 * (c + 1)]
                            for r in range(NCHUNK)
                        ]
                        _banded_mm(
                            nc, pv[:, N * c : N * (c + 1)], lhsT,
                            w_sl[wname], RR[wname],
                        )
                    nc.scalar.activation(tq[:], pv[:], CPY)
                    tT[k] = tq

                def lhs(key, rt):
                    row0 = 128 * rt
                    return [
                        tT[key][:, N * c + row0 : N * c + row0 + 128]
                        for c in range(NCHUNK)
                    ]

                Sq = post.tile([128, NCHUNK * W2], F16, tag="S")
                S3 = Sq[:].rearrange("p (q w) -> p q w", w=W2)
                nc.gpsimd.memset(S3[:, :, 0:1], 0.0)
                nc.gpsimd.memset(S3[:, :, N + 1 : N + 2], 0.0)
                sxT = post.tile([128, NCHUNK * N], F16, tag="sxT")
                syq = post.tile([128, NCHUNK * N], F16, tag="sy")
                ssq = post.tile([128, NCHUNK * N], F16, tag="ss")
                Upq = post.tile([128, NCHUNK * W2], F16, tag="Up")
                Dnq = post.tile([128, NCHUNK * W2], F16, tag="Dn")
                U3 = Upq[:].rearrange("p (q w) -> p q w", w=W2)
                D3 = Dnq[:].rearrange("p (q w) -> p q w", w=W2)
                zr3 = zrow[:].rearrange("p (q w) -> p q w", w=W2)
                TT = nc.vector.tensor_tensor

                # --- blur pipeline (sobel-x vertical interleaved so the
                # mask pipeline's PE work overlaps the blur/pairs stage) ---
                vpass("b", "bv")
                vpass("x", "mvx")
                for rt in range(NCHUNK):
                    pb = psb.tile([128, N], F32, tag="pb")
                    _banded_mm(nc, pb, lhs("b", rt), w_sl["bh"], r5)
                    q0 = rt * W2
                    nc.scalar.activation(
                        Sq[:, q0 + 1 : q0 + N + 1], pb[:], CPY, scale=s
                    )

                # --- Up/Dn partition shifts + pair maxes per half ---
                pairs = []
                for h in range(2):
                    qs = slice(2 * h, 2 * h + 2)
                    # Up[p, q] = S[p+1, q]; Up[127, q] = S[0, q+1] (0 at q=3)
                    nc.sync.dma_start(out=U3[0:127, qs, :], in_=S3[1:128, qs, :])
                    if h == 0:
                        nc.sync.dma_start(
                            out=U3[127:128, 0:2, :], in_=S3[0:1, 1:3, :]
                        )
                    else:
                        nc.sync.dma_start(
                            out=U3[127:128, 2:3, :], in_=S3[0:1, 3:4, :]
                        )
                        nc.sync.dma_start(out=U3[127:128, 3:4, :], in_=zr3)
                    # Dn[p, q] = S[p-1, q]; Dn[0, q] = S[127, q-1] (0 at q=0)
                    nc.sync.dma_start(out=D3[1:128, qs, :], in_=S3[0:127, qs, :])
                    if h == 0:
                        nc.sync.dma_start(out=D3[0:1, 0:1, :], in_=zr3)
                        nc.sync.dma_start(
                            out=D3[0:1, 1:2, :], in_=S3[127:128, 0:1, :]
                        )
                    else:
                        nc.sync.dma_start(
                            out=D3[0:1, 2:4, :], in_=S3[127:128, 1:3, :]
                        )

                for h in range(2):
                    qs = slice(2 * h, 2 * h + 2)
                    S3h, U3h, D3h = S3[:, qs, :], U3[:, qs, :], D3[:, qs, :]
                    selq = post.tile([128, 2 * N], F16, tag=f"sel{h}")
                    sel3 = selq[:].rearrange("p (q w) -> p q w", w=N)
                    paq = post.tile([128, 2 * N], F16, tag=f"panti{h}")
                    pa3 = paq[:].rearrange("p (q w) -> p q w", w=N)
                    p2q = post.tile([128, 2 * N], F16, tag=f"pm2{h}")
                    p23 = p2q[:].rearrange("p (q w) -> p q w", w=N)
                    p0q = post.tile([128, 2 * N], F16, tag=f"pm0{h}")
                    p03 = p0q[:].rearrange("p (q w) -> p q w", w=N)
                    TT(out=p03, in0=S3h[:, :, 0:N], in1=S3h[:, :, 2 : N + 2],
                       op=mybir.AluOpType.max)       # horizontal {Sl, Sr}
                    TT(out=sel3, in0=D3h[:, :, 0:N], in1=U3h[:, :, 2 : N + 2],
                       op=mybir.AluOpType.max)       # main diag {UL, DR}
                    TT(out=pa3, in0=D3h[:, :, 2 : N + 2], in1=U3h[:, :, 0:N],
                       op=mybir.AluOpType.max)       # anti diag {UR, DL}
                    TT(out=p23, in0=D3h[:, :, 1 : N + 1], in1=U3h[:, :, 1 : N + 1],
                       op=mybir.AluOpType.max)       # vertical {Uc, Dc}
                    pairs.append((selq, paq, p2q, p0q))

                # --- sobel pipelines ---
                for rt in range(NCHUNK):
                    pgx = psx.tile([128, N], F32, tag="pgx")
                    _banded_mm(nc, pgx, lhs("x", rt), w_sl["mhx"], r7)
                    f0 = rt * N
                    nc.scalar.activation(sxT[:, f0 : f0 + N], pgx[:], SQ, scale=T1)
                vpass("y", "mvy")
                for rt in range(NCHUNK):
                    # pgy accumulation stays open: after gy^2 extract, one
                    # more sweep adds gx in-bank for (gx+gy)^2.
                    pgy = psy.tile([128, N], F32, tag="pgy")
                    _banded_mm(nc, pgy, lhs("y", rt), w_sl["mhy"], r7, stop=False)
                    f0 = rt * N
                    nc.scalar.activation(syq[:, f0 : f0 + N], pgy[:], SQ)
                    for r in range(NCHUNK):
                        lo2, hi2 = r7[r]
                        wap, lo0 = w_sl["mhx"][r]
                        nc.tensor.matmul(
                            pgy[:, lo2:hi2],
                            lhs("x", rt)[r],
                            wap[:, lo2 - lo0 : hi2 - lo0],
                            start=False,
                            stop=(r == NCHUNK - 1),
                        )
                    nc.scalar.activation(ssq[:, f0 : f0 + N], pgy[:], SQ)

                # --- per-half masks + select + gate tail ---
                for h in range(2):
                    fh = slice(2 * h * N, (2 * h + 2) * N)
                    qs = slice(2 * h, 2 * h + 2)
                    S3h = S3[:, qs, :]
                    selq, paq, p2q, p0q = pairs[h]

                    # masks (Pool rescales off critical path + DVE TT 2x)
                    syT = post.tile([128, 2 * N], F16, tag=f"syT{h}")
                    sxT2 = post.tile([128, 2 * N], F16, tag=f"sxT2{h}")
                    nc.gpsimd.tensor_scalar(
                        out=syT[:], in0=syq[:, fh], scalar1=T1 ** 4,
                        scalar2=None, op0=mybir.AluOpType.mult,
                    )
                    nc.gpsimd.tensor_scalar(
                        out=sxT2[:], in0=sxT[:, fh], scalar1=2.0 / T1 ** 2,
                        scalar2=None, op0=mybir.AluOpType.mult,
                    )
                    m0q = post.tile([128, 2 * N], F16, tag=f"m0{h}")
                    m2q = post.tile([128, 2 * N], F16, tag=f"m2{h}")
                    mnq = post.tile([128, 2 * N], F16, tag=f"mneg{h}")
                    TT(out=mnq[:], in0=sxT2[:], in1=ssq[:, fh],
                       op=mybir.AluOpType.is_gt)
                    TT(out=m2q[:], in0=syT[:], in1=sxT[:, fh],
                       op=mybir.AluOpType.is_gt)
                    TT(out=m0q[:], in0=sxT[:, fh], in1=syq[:, fh],
                       op=mybir.AluOpType.is_ge)

                    nc.vector.copy_predicated(selq[:], mnq[:].bitcast(U16), paq[:])
                    nc.vector.copy_predicated(selq[:], m2q[:].bitcast(U16), p2q[:])
                    nc.vector.copy_predicated(selq[:], m0q[:].bitcast(U16), p0q[:])

                    # out = relu(x*(S - sel/zs))*zs  (x >= 0)
                    hq = post.tile([128, 2 * N], F16, tag=f"hsel{h}")
                    zq = post.tile([128, 2 * N], F16, tag=f"z{h}")
                    wq2 = post.tile([128, 2 * N], F16, tag=f"wz{h}")
                    oq = outp.tile([128, 2 * N], F16, tag=f"o{h}")
                    nc.vector.tensor_scalar(
                        out=hq[:], in0=selq[:], scalar1=1.0 / zs, scalar2=None,
                        op0=mybir.AluOpType.mult,
                    )
                    z3 = zq[:].rearrange("p (q w) -> p q w", w=N)
                    TT(out=z3, in0=S3h[:, :, 1 : N + 1],
                       in1=hq[:].rearrange("p (q w) -> p q w", w=N),
                       op=mybir.AluOpType.subtract)
                    TT(out=wq2[:], in0=zq[:], in1=xq[:, fh],
                       op=mybir.AluOpType.mult)
                    nc.vector.tensor_scalar(
                        out=oq[:], in0=wq2[:], scalar1=0.0, scalar2=zs,
                        op0=mybir.AluOpType.max, op1=mybir.AluOpType.mult,
                    )

                    # store this half, 1 desc
                    nc.gpsimd.dma_start(
                        out=out_d[
                            i * N + 256 * h : i * N + 256 * (h + 1), :
                        ].rearrange("(q p) w -> p q w", p=128),
                        in_=oq[:].rearrange("p (q w) -> p q w", w=N),
                    )

    nc.compile()
    return nc


# ---------------------------------------------------------------------------
# host side
# ---------------------------------------------------------------------------

def _make_band(weights, offsets, pad):
    M = np.zeros((N, N), dtype=np.float64)
    for w, o in zip(weights, offsets):
        idx = np.arange(N)
        src = idx + o
        if pad == "replicate":
            np.add.at(M, (np.clip(src, 0, N - 1), idx), w)
        else:
            ok = (src >= 0) & (src < N)
            np.add.at(M, (src[ok], idx[ok]), w)
    return M


def _host_weights(gauss_kernel):
    gk = np.asarray(gauss_kernel, dtype=np.float64)[0, 0]
    U, sv, Vt = np.linalg.svd(gk)
    assert sv[1] < 1e-5 * sv[0], "gauss kernel not rank-1 separable"
    wv = U[:, 0] * np.sqrt(sv[0])
    wh = Vt[0] * np.sqrt(sv[0])
    if wv.sum() < 0:
        wv, wh = -wv, -wh
    o5 = [-2, -1, 0, 1, 2]
    o3 = [-1, 0, 1]
    Bv = _make_band(wv, o5, "zero")
    Bh = _make_band(wh, o5, "zero")
    Sv = _make_band([1, 2, 1], o3, "replicate")
    Dv = _make_band([-1, 0, 1], o3, "replicate")
    Sh = _make_band([1, 2, 1], o3, "replicate")
    Dh = _make_band([-1, 0, 1], o3, "replicate")
    mats = {
        "bv": Bv, "bh": Bh,
        "mvx": Bv @ Sv, "mhx": Bh @ Dh,
        "mvy": Bv @ Dv, "mhy": Bh @ Sh,
    }
    halo = {"bv": 2, "bh": 2, "mvx": 3, "mhx": 3, "mvy": 3, "mhy": 3}
    wq = np.zeros((128, len(W_NAMES) * NCHUNK * WBAND), dtype=np.float16)
    for ki, k in enumerate(W_NAMES):
        M = mats[k]
        h = halo[k]
        for r in range(NCHUNK):
            lo = max(0, 128 * r - h)
            hi = min(N, 128 * r + 128 + h)
            c0 = (ki * NCHUNK + r) * WBAND
            wq[:, c0 : c0 + (hi - lo)] = M[128 * r : 128 * (r + 1), lo:hi].astype(
                np.float16
            )
    return wq


_NC_CACHE = {}
LAST_RESULT = None


def kernel(reconst, gauss_kernel, nms_kernel):
    nk = np.asarray(nms_kernel, dtype=np.float64)
    cen = float(nk[0, 0, 1, 1])
    v = float(nk[0, 0, 1, 2])
    # verify nms kernel structure: center + single tap v per direction
    pos = [(1, 2), (2, 2), (2, 1), (2, 0), (1, 0), (0, 0), (0, 1), (0, 2)]
    for d, (r, c) in enumerate(pos):
        k = nk[d, 0].copy()
        assert abs(k[1, 1] - cen) < 1e-6 and abs(k[r, c] - v) < 1e-6
        k[1, 1] = 0.0
        k[r, c] = 0.0
        assert np.abs(k).max() < 1e-7
    assert v < 0

    key = (round(cen, 9), round(v, 9))
    if key not in _NC_CACHE:
        _NC_CACHE[key] = build_nc(cen, v)
    nc = _NC_CACHE[key]

    wq = _host_weights(gauss_kernel)
    x = np.asarray(reconst, dtype=np.float32).reshape(B_TOTAL, N, N)
    in_maps = []
    for core in range(N_CORES):
        m = {
            "x": np.ascontiguousarray(
                x[core * B_CORE : (core + 1) * B_CORE].reshape(B_CORE * N, N)
            ),
            "wq": wq,
        }
        in_maps.append(m)

    res = run_bass_kernel_spmd(nc, in_maps, core_ids=list(range(N_CORES)))
    global LAST_RESULT
    LAST_RESULT = res
    out = np.concatenate(
        [r["out"].reshape(B_CORE, 1, N, N) for r in res.results], axis=0
    )
    return out.astype(np.float32)
